# revision 37
# baseline (speedup 1.0000x reference)
"""CompGCN layer kernel for 8 Trainium2 NeuronCores.

Strategy (dst-sharded, gather + selector-matmul aggregation, no collectives):
  - Each core owns 6250 destination nodes and receives exactly the edges
    whose dst falls in its range (host bucketing).
  - (h[src] + rel[type]) @ W == (h@W)[src] + (rel@W)[type]. Each core builds
    hW = h @ W_neighbor (fp16 table, replicated) on the tensor engine.
  - Edges are grouped per dst-tile (128 dst nodes) into two statically-sized
    segments: A (src < 32768) and B (src >= 32768, local idx) — int16 gather
    indices can't span 50048 rows. GPSIMD dma_gather pulls hW[src] for each
    segment (edge-major fp16 tiles).
  - Aggregation is a matmul: for each 128-edge K-tile, a one-hot selector
    S[e, j] = (dst_rel_e == j) (built on DVE via iota + tensor_scalar
    is_equal) is the stationary operand; msg tiles are moving; PSUM [dst, d]
    accumulates all K-tiles of the dst-tile. Pad edges carry dst_rel = -1 so
    their selector column is all-zero — pads are free.
  - rel contribution folds in exactly as C^T @ relW (C = per-core dst x type
    count histogram, host-built, fp16-exact) accumulated into the same PSUM.
  - Fused final: out = relu((agg) * norm + h_slice @ loop_weight) per tile.
  - dma_gather HW limits: <= 1024 idxs per op is the empirically stable size
    (the SWDGE desc rings overflow beyond that; crash, not slowdown).
"""

import os
import math
import numpy as np
from dataclasses import dataclass
from contextlib import ExitStack

from concourse import bacc, bass, mybir, tile
from concourse.bass_utils import run_bass_kernel_spmd
from concourse.masks import make_identity

F32 = mybir.dt.float32
F16 = mybir.dt.float16
I16 = mybir.dt.int16

GCHUNK = int(os.environ.get("KERNEL_GCHUNK", "1024"))  # idxs per dma_gather


@dataclass(frozen=True)
class Cfg:
    n_nodes: int = 50000
    d: int = 128
    n_rels: int = 500
    n_cores: int = 8
    split: int = 32768
    n_edges: int = 600000

    @property
    def np_core(self):
        return self.n_nodes // self.n_cores

    @property
    def np_pad(self):
        return ((self.np_core + 1 + 127) // 128) * 128

    @property
    def n_pad(self):
        return ((self.n_nodes + 127) // 128) * 128

    @property
    def r_pad(self):
        return ((self.n_rels + 127) // 128) * 128

    def seg_size(self, table: str) -> int:
        """Static per-dst-tile segment size (multiple of 128) for stream A/B.

        Edge count of one dst-tile (128 dsts) in one stream is ~Poisson(mean);
        size at mean + 6.5 sigma, rounded up to 128."""
        lam_node = self.n_edges / self.n_nodes
        share = (self.split / self.n_nodes if table == "A"
                 else (self.n_nodes - self.split) / self.n_nodes)
        mean = 128 * lam_node * share
        sz = mean + 6.5 * math.sqrt(mean) + 16
        return int(math.ceil(sz / 128.0)) * 128


CFG = Cfg()


def build_program(cfg: Cfg, debug_outputs: bool = False):
    nc = bacc.Bacc("TRN2", target_bir_lowering=False, debug=False, num_swdge_queues=4)
    D = cfg.d
    na, nb = cfg.seg_size("A"), cfg.seg_size("B")
    seg = na + nb                       # edges per dst-tile segment
    n_slice_tiles = cfg.np_pad // 128
    n_h_tiles = cfg.n_pad // 128
    n_r_tiles = cfg.r_pad // 128
    total_e = seg * n_slice_tiles       # padded edge stream per core
    n_ktiles = seg // 128

    h_full = nc.dram_tensor("h_full", [cfg.n_pad, D], F32, kind="ExternalInput")
    rel_emb = nc.dram_tensor("rel_emb", [cfg.r_pad, D], F32, kind="ExternalInput")
    w_nb = nc.dram_tensor("w_nb", [D, D], F32, kind="ExternalInput")
    w_loop = nc.dram_tensor("w_loop", [D, D], F32, kind="ExternalInput")
    h_slice = nc.dram_tensor("h_slice", [cfg.np_pad, D], F32, kind="ExternalInput")
    norm_cols = nc.dram_tensor("norm_cols", [D, n_slice_tiles], F32, kind="ExternalInput")
    ct_hist = nc.dram_tensor("ct_hist", [cfg.r_pad, cfg.np_pad], F16, kind="ExternalInput")
    src_idx = nc.dram_tensor("src_idx", [128, total_e // 16], I16, kind="ExternalInput")
    dst_rel = nc.dram_tensor("dst_rel", [128, total_e // 128], F32, kind="ExternalInput")

    tab_lo = nc.dram_tensor("tab_lo", [cfg.split, D], F16)
    tab_hi = nc.dram_tensor("tab_hi", [cfg.n_pad - cfg.split, D], F16)
    out = nc.dram_tensor("out", [cfg.np_pad, D], F32, kind="ExternalOutput")
    if debug_outputs:
        dbg_lo = nc.dram_tensor("dbg_lo", [cfg.split, D], F16, kind="ExternalOutput")
        dbg_hi = nc.dram_tensor("dbg_hi", [cfg.n_pad - cfg.split, D], F16, kind="ExternalOutput")

    with tile.TileContext(nc) as tc:
        with ExitStack() as ex:
            cpool = ex.enter_context(tc.tile_pool(name="const", bufs=1))
            inpool = ex.enter_context(tc.tile_pool(name="ld", bufs=4))
            ptpool = ex.enter_context(tc.tile_pool(name="psT", bufs=2, space="PSUM"))
            pwpool = ex.enter_context(tc.tile_pool(name="psW", bufs=3, space="PSUM"))
            agpool = ex.enter_context(tc.tile_pool(name="psA", bufs=3, space="PSUM"))
            stpool = ex.enter_context(tc.tile_pool(name="sbT", bufs=4))
            swpool = ex.enter_context(tc.tile_pool(name="sbW", bufs=4))
            segpool = ex.enter_context(tc.tile_pool(name="seg", bufs=4))
            selpool = ex.enter_context(tc.tile_pool(name="sel", bufs=6))
            fpool = ex.enter_context(tc.tile_pool(name="fin", bufs=4))

            # ------- constants -------
            ident = cpool.tile([128, 128], F32)
            make_identity(nc, ident[:])
            ident16 = cpool.tile([128, 128], F16)
            make_identity(nc, ident16[:])
            iota_row = cpool.tile([128, 128], F32)
            nc.gpsimd.iota(iota_row[:], pattern=[[1, 128]], base=0,
                           channel_multiplier=0,
                           allow_small_or_imprecise_dtypes=True)
            w_sb = cpool.tile([128, D], F32)
            nc.sync.dma_start(out=w_sb[:], in_=w_nb[:, :])
            wl_sb = cpool.tile([128, D], F32)
            nc.sync.dma_start(out=wl_sb[:], in_=w_loop[:, :])
            norm_sb = cpool.tile([128, n_slice_tiles], F32)
            nc.sync.dma_start(out=norm_sb[:], in_=norm_cols[:, :])
            src_sb = cpool.tile([128, total_e // 16], I16)
            nc.sync.dma_start(out=src_sb[:], in_=src_idx[:, :])
            dc_sb = cpool.tile([128, total_e // 128], F32)
            nc.sync.dma_start(out=dc_sb[:], in_=dst_rel[:, :])

            # ------- precompute hW/relW tables on PE (fp16 math, 4-tile batches) -------
            def xw_group(src_dram, t0, nt, w16, sinks):
                """sinks: list of (dst_dram, row0) destinations for the fp16 result."""
                x4 = inpool.tile([128, 4, D], F32, tag="x4")
                src_ap = src_dram[t0 * 128:(t0 + nt) * 128, :]
                nc.sync.dma_start(out=x4[:, 0:nt, :],
                                  in_=src_ap.rearrange("(b p) d -> p b d", p=128))
                x16 = inpool.tile([128, 4, D], F16, tag="x16")
                nc.vector.tensor_copy(out=x16[:, 0:nt, :], in_=x4[:, 0:nt, :])
                sw4 = swpool.tile([128, 4, D], F16, tag="sw4")
                for b in range(nt):
                    pt = ptpool.tile([128, 128], F16, tag="pt")
                    nc.tensor.transpose(out=pt[:], in_=x16[:, b, :], identity=ident16[:])
                    st = stpool.tile([128, 128], F16)
                    nc.vector.tensor_copy(out=st[:], in_=pt[:])
                    pw = pwpool.tile([128, D], F32)
                    nc.tensor.matmul(out=pw[:], lhsT=st[:], rhs=w16[:],
                                     start=True, stop=True)
                    nc.vector.tensor_copy(out=sw4[:, b, :], in_=pw[:])
                for dst_dram, row0 in sinks:
                    dst_ap = dst_dram[row0:row0 + nt * 128, :]
                    nc.sync.dma_start(out=dst_ap.rearrange("(b p) d -> p b d", p=128),
                                      in_=sw4[:, 0:nt, :])

            w16 = cpool.tile([128, D], F16)
            nc.vector.tensor_copy(out=w16[:], in_=w_sb[:])

            # relW stays in SBUF (fp16) for the C^T @ relW matmuls
            relw16 = cpool.tile([128, n_r_tiles, D], F16)
            for t0 in range(0, n_r_tiles, 4):
                nt = min(4, n_r_tiles - t0)
                x4 = inpool.tile([128, 4, D], F32, tag="x4")
                nc.sync.dma_start(
                    out=x4[:, 0:nt, :],
                    in_=rel_emb[t0 * 128:(t0 + nt) * 128, :].rearrange(
                        "(b p) d -> p b d", p=128))
                x16 = inpool.tile([128, 4, D], F16, tag="x16")
                nc.vector.tensor_copy(out=x16[:, 0:nt, :], in_=x4[:, 0:nt, :])
                for b in range(nt):
                    pt = ptpool.tile([128, 128], F16, tag="pt")
                    nc.tensor.transpose(out=pt[:], in_=x16[:, b, :], identity=ident16[:])
                    st = stpool.tile([128, 128], F16)
                    nc.vector.tensor_copy(out=st[:], in_=pt[:])
                    pw = pwpool.tile([128, D], F32)
                    nc.tensor.matmul(out=pw[:], lhsT=st[:], rhs=w16[:],
                                     start=True, stop=True)
                    nc.vector.tensor_copy(out=relw16[:, t0 + b, :], in_=pw[:])

            n_lo_tiles = cfg.split // 128
            for t0 in range(0, n_lo_tiles, 4):
                xw_group(h_full, t0, min(4, n_lo_tiles - t0), w16, [(tab_lo, t0 * 128)])
            for t0 in range(n_lo_tiles, n_h_tiles, 4):
                nt = min(4, n_h_tiles - t0)
                xw_group(h_full, t0, nt, w16, [(tab_hi, t0 * 128 - cfg.split)])

            # ------- per dst-tile: gather + selector matmuls + fused finish -------
            qctr = [0]

            def nextq():
                qctr[0] += 1
                return qctr[0] % 4

            for t in range(n_slice_tiles):
                base = t * seg
                st16 = segpool.tile([128, n_ktiles, D], F16, tag="seg")
                for part_base, part_len, tab in ((0, na, tab_lo), (na, nb, tab_hi)):
                    for c0 in range(0, part_len, GCHUNK):
                        n = min(GCHUNK, part_len - c0)
                        o = base + part_base + c0
                        nc.gpsimd.dma_gather(
                            out_ap=st16[:, (part_base + c0) // 128:(part_base + c0 + n) // 128, :],
                            in_ap=tab[:, :], idxs_ap=src_sb[:, o // 16:(o + n) // 16],
                            num_idxs=n, num_idxs_reg=n, elem_size=D, queue_num=nextq())

                agg = agpool.tile([128, D], F32, tag="agg")
                nmm = n_ktiles + n_r_tiles
                k = 0
                for kt in range(n_ktiles):
                    sel = selpool.tile([128, 128], F16, tag="sel")
                    nc.vector.tensor_scalar(
                        out=sel[:], in0=iota_row[:],
                        scalar1=dc_sb[:, base // 128 + kt:base // 128 + kt + 1],
                        scalar2=None, op0=mybir.AluOpType.is_equal)
                    nc.tensor.matmul(out=agg[:], lhsT=sel[:], rhs=st16[:, kt, :],
                                     start=(k == 0), stop=(k == nmm - 1))
                    k += 1
                # rel contribution: C^T.T @ relW accumulated into the same PSUM
                ct = fpool.tile([128, n_r_tiles, 128], F16, tag="ct_ld")
                nc.sync.dma_start(
                    out=ct[:],
                    in_=ct_hist[:, t * 128:(t + 1) * 128].rearrange(
                        "(rt r) d -> r rt d", r=128))
                for rt in range(n_r_tiles):
                    nc.tensor.matmul(out=agg[:], lhsT=ct[:, rt, :], rhs=relw16[:, rt, :],
                                     start=(k == 0), stop=(k == nmm - 1))
                    k += 1

                # loop message
                hs = inpool.tile([128, D], F32, tag="hs")
                nc.sync.dma_start(out=hs[:], in_=h_slice[t * 128:(t + 1) * 128, :])
                pt = ptpool.tile([128, 128], F32, tag="pt")
                nc.tensor.transpose(out=pt[:], in_=hs[:], identity=ident[:])
                stl = stpool.tile([128, 128], F32)
                nc.vector.tensor_copy(out=stl[:], in_=pt[:])
                pl = pwpool.tile([128, D], F32, tag="pw")
                nc.tensor.matmul(out=pl[:], lhsT=stl[:], rhs=wl_sb[:], start=True, stop=True)

                m = fpool.tile([128, D], F32)
                nc.vector.tensor_scalar(out=m[:], in0=agg[:], scalar1=norm_sb[:, t:t + 1],
                                        scalar2=None, op0=mybir.AluOpType.mult)
                nc.vector.tensor_add(out=m[:], in0=m[:], in1=pl[:])
                nc.vector.tensor_scalar_max(out=m[:], in0=m[:], scalar1=0.0)
                nc.sync.dma_start(out=out[t * 128:(t + 1) * 128, :], in_=m[:])

            if debug_outputs:
                def dbg_copy(src, dst, ntiles):
                    for tt in range(ntiles):
                        dd = fpool.tile([128, D], F16, tag="dbg")
                        nc.sync.dma_start(out=dd[:], in_=src[tt * 128:(tt + 1) * 128, :])
                        nc.sync.dma_start(out=dst[tt * 128:(tt + 1) * 128, :], in_=dd[:])
                dbg_copy(tab_lo, dbg_lo, cfg.split // 128)
                dbg_copy(tab_hi, dbg_hi, (cfg.n_pad - cfg.split) // 128)

    nc.compile()
    return nc


def _wrap16(vals: np.ndarray, pad_len: int, pad_val: int) -> np.ndarray:
    a = np.full(pad_len, pad_val, dtype=np.int16)
    a[:len(vals)] = vals.astype(np.int16)
    w16 = a.reshape(pad_len // 16, 16).T
    return np.tile(w16, (8, 1)).copy()


def prep_inputs(cfg: Cfg, h, norm, rel_emb, w_nb, w_loop, edge_src, edge_dst, edge_type):
    h = np.asarray(h, np.float32)
    norm = np.asarray(norm, np.float32).reshape(-1)
    rel_emb = np.asarray(rel_emb, np.float32)
    edge_src = np.asarray(edge_src, np.int64)
    edge_dst = np.asarray(edge_dst, np.int64)
    edge_type = np.asarray(edge_type, np.int64)

    na, nb = cfg.seg_size("A"), cfg.seg_size("B")
    seg = na + nb
    n_slice_tiles = cfg.np_pad // 128
    total_e = seg * n_slice_tiles

    h_pad = np.zeros((cfg.n_pad, cfg.d), np.float32)
    h_pad[:cfg.n_nodes] = h
    r_pad = np.zeros((cfg.r_pad, cfg.d), np.float32)
    r_pad[:cfg.n_rels] = rel_emb

    in_maps = []
    for c in range(cfg.n_cores):
        lo, hi = c * cfg.np_core, (c + 1) * cfg.np_core
        sel = (edge_dst >= lo) & (edge_dst < hi)
        src_c, dst_c, typ_c = edge_src[sel], edge_dst[sel] - lo, edge_type[sel]

        src_stream = np.zeros(total_e, np.int64)
        dc_stream = np.full(total_e, -1.0, np.float32)
        dtile = dst_c // 128
        in_a = src_c < cfg.split
        for t in range(n_slice_tiles):
            tm = dtile == t
            for part0, plen, pm, soff in ((0, na, tm & in_a, 0),
                                          (na, nb, tm & ~in_a, cfg.split)):
                s = src_c[pm] - soff
                d = dst_c[pm] - t * 128
                assert len(s) <= plen, (c, t, len(s), plen)
                o = np.argsort(s, kind="stable")
                base = t * seg + part0
                src_stream[base:base + len(s)] = s[o]
                dc_stream[base:base + len(s)] = d[o]

        hsl = np.zeros((cfg.np_pad, cfg.d), np.float32)
        hsl[:cfg.np_core] = h[lo:hi]
        ntmp = np.zeros(cfg.np_pad, np.float32)
        ntmp[:cfg.np_core] = norm[lo:hi]
        ncol = np.ascontiguousarray(ntmp.reshape(n_slice_tiles, cfg.d).T)

        ct = np.zeros((cfg.r_pad, cfg.np_pad), np.float32)
        np.add.at(ct, (typ_c, dst_c), 1.0)

        in_maps.append({
            "h_full": h_pad, "rel_emb": r_pad,
            "w_nb": np.asarray(w_nb, np.float32), "w_loop": np.asarray(w_loop, np.float32),
            "h_slice": hsl, "norm_cols": ncol,
            "ct_hist": ct.astype(np.float16),
            "src_idx": _wrap16(src_stream, total_e, 0),
            "dst_rel": np.ascontiguousarray(
                dc_stream.reshape(total_e // 128, 128).T),
        })
    return in_maps


_CACHED = {}


def _get_program(cfg: Cfg):
    if cfg not in _CACHED:
        _CACHED[cfg] = build_program(cfg)
    return _CACHED[cfg]


LAST_RESULTS = None


def kernel(h, norm, rel_emb, W_neighbor, loop_weight, edge_src, edge_dst, edge_type):
    cfg = CFG
    nc = _get_program(cfg)
    in_maps = prep_inputs(cfg, h, norm, rel_emb, W_neighbor, loop_weight,
                          edge_src, edge_dst, edge_type)
    trace = os.environ.get("KERNEL_TRACE", "0") == "1"
    res = run_bass_kernel_spmd(nc, in_maps, list(range(cfg.n_cores)), trace=trace)
    global LAST_RESULTS
    LAST_RESULTS = res
    outs = [res.results[c]["out"][:cfg.np_core] for c in range(cfg.n_cores)]
    return np.concatenate(outs, axis=0).astype(np.float32)


# revision 39
# speedup vs baseline: 1.2779x; 1.2779x over previous
"""CompGCN layer kernel for 8 Trainium2 NeuronCores.

Strategy (dst-sharded, gather + selector-matmul aggregation, no collectives):
  - Each core owns 6250 destination nodes and receives exactly the edges
    whose dst falls in its range (host bucketing).
  - (h[src] + rel[type]) @ W == (h@W)[src] + (rel@W)[type]. Each core builds
    hW = h @ W_neighbor (fp16 table, replicated) on the tensor engine.
  - Edges are grouped per dst-tile (128 dst nodes) into two statically-sized
    segments: A (src < 32768) and B (src >= 32768, local idx) — int16 gather
    indices can't span 50048 rows. GPSIMD dma_gather pulls hW[src] for each
    segment (edge-major fp16 tiles).
  - Aggregation is a matmul: for each 128-edge K-tile, a one-hot selector
    S[e, j] = (dst_rel_e == j) (built on DVE via iota + tensor_scalar
    is_equal) is the stationary operand; msg tiles are moving; PSUM [dst, d]
    accumulates all K-tiles of the dst-tile. Pad edges carry dst_rel = -1 so
    their selector column is all-zero — pads are free.
  - rel contribution folds in exactly as C^T @ relW (C = per-core dst x type
    count histogram, host-built, fp16-exact) accumulated into the same PSUM.
  - Fused final: out = relu((agg) * norm + h_slice @ loop_weight) per tile.
  - dma_gather HW limits: <= 1024 idxs per op is the empirically stable size
    (the SWDGE desc rings overflow beyond that; crash, not slowdown).
"""

import os
import math
import numpy as np
from dataclasses import dataclass
from contextlib import ExitStack

from concourse import bacc, bass, mybir, tile
from concourse.bass_utils import run_bass_kernel_spmd
from concourse.masks import make_identity

F32 = mybir.dt.float32
F16 = mybir.dt.float16
I16 = mybir.dt.int16

GCHUNK = int(os.environ.get("KERNEL_GCHUNK", "1024"))  # idxs per dma_gather


@dataclass(frozen=True)
class Cfg:
    n_nodes: int = 50000
    d: int = 128
    n_rels: int = 500
    n_cores: int = 8
    split: int = 32768
    n_edges: int = 600000

    @property
    def np_core(self):
        return self.n_nodes // self.n_cores

    @property
    def np_pad(self):
        return ((self.np_core + 1 + 127) // 128) * 128

    @property
    def n_pad(self):
        return ((self.n_nodes + 127) // 128) * 128

    @property
    def r_pad(self):
        return ((self.n_rels + 127) // 128) * 128

    def seg_size(self, table: str) -> int:
        """Static per-dst-tile segment size (multiple of 128) for stream A/B.

        Edge count of one dst-tile (128 dsts) in one stream is ~Poisson(mean);
        size at mean + 6.5 sigma, rounded up to 128."""
        lam_node = self.n_edges / self.n_nodes
        share = (self.split / self.n_nodes if table == "A"
                 else (self.n_nodes - self.split) / self.n_nodes)
        mean = 128 * lam_node * share
        sz = mean + 6.5 * math.sqrt(mean) + 16
        return int(math.ceil(sz / 128.0)) * 128


CFG = Cfg()


def build_program(cfg: Cfg, debug_outputs: bool = False):
    nc = bacc.Bacc("TRN2", target_bir_lowering=False, debug=False, num_swdge_queues=4)
    D = cfg.d
    na, nb = cfg.seg_size("A"), cfg.seg_size("B")
    seg = na + nb                       # edges per dst-tile segment
    n_slice_tiles = cfg.np_pad // 128
    n_h_tiles = cfg.n_pad // 128
    n_r_tiles = cfg.r_pad // 128
    total_e = seg * n_slice_tiles       # padded edge stream per core
    n_ktiles = seg // 128

    h_t = nc.dram_tensor("h_t", [D, cfg.n_pad], F16, kind="ExternalInput")
    rel_t = nc.dram_tensor("rel_t", [D, cfg.r_pad], F16, kind="ExternalInput")
    w_nb = nc.dram_tensor("w_nb", [D, D], F32, kind="ExternalInput")
    w_loop = nc.dram_tensor("w_loop", [D, D], F32, kind="ExternalInput")
    hs_t = nc.dram_tensor("hs_t", [D, cfg.np_pad], F16, kind="ExternalInput")
    norm_cols = nc.dram_tensor("norm_cols", [D, n_slice_tiles], F32, kind="ExternalInput")
    ct_hist = nc.dram_tensor("ct_hist", [cfg.r_pad, cfg.np_pad], F16, kind="ExternalInput")
    src_idx = nc.dram_tensor("src_idx", [128, total_e // 16], I16, kind="ExternalInput")
    dst_rel = nc.dram_tensor("dst_rel", [128, total_e // 128], F32, kind="ExternalInput")

    tab_lo = nc.dram_tensor("tab_lo", [cfg.split, D], F16)
    tab_hi = nc.dram_tensor("tab_hi", [cfg.n_pad - cfg.split, D], F16)
    out = nc.dram_tensor("out", [cfg.np_pad, D], F32, kind="ExternalOutput")
    if debug_outputs:
        dbg_lo = nc.dram_tensor("dbg_lo", [cfg.split, D], F16, kind="ExternalOutput")
        dbg_hi = nc.dram_tensor("dbg_hi", [cfg.n_pad - cfg.split, D], F16, kind="ExternalOutput")

    with tile.TileContext(nc) as tc:
        with ExitStack() as ex:
            cpool = ex.enter_context(tc.tile_pool(name="const", bufs=1))
            inpool = ex.enter_context(tc.tile_pool(name="ld", bufs=4))
            pwpool = ex.enter_context(tc.tile_pool(name="psW", bufs=3, space="PSUM"))
            agpool = ex.enter_context(tc.tile_pool(name="psA", bufs=4, space="PSUM"))
            swpool = ex.enter_context(tc.tile_pool(name="sbW", bufs=4))
            segpool = ex.enter_context(tc.tile_pool(name="seg", bufs=4))
            selpool = ex.enter_context(tc.tile_pool(name="sel", bufs=6))
            fpool = ex.enter_context(tc.tile_pool(name="fin", bufs=4))

            # ------- constants -------
            iota_big = cpool.tile([128, n_ktiles, 128], F32)
            nc.gpsimd.iota(iota_big[:], pattern=[[0, n_ktiles], [1, 128]], base=0,
                           channel_multiplier=0,
                           allow_small_or_imprecise_dtypes=True)
            w_sb = cpool.tile([128, D], F32)
            nc.sync.dma_start(out=w_sb[:], in_=w_nb[:, :])
            w16 = cpool.tile([128, D], F16)
            nc.vector.tensor_copy(out=w16[:], in_=w_sb[:])
            wl_sb = cpool.tile([128, D], F32)
            nc.sync.dma_start(out=wl_sb[:], in_=w_loop[:, :])
            wl16 = cpool.tile([128, D], F16)
            nc.vector.tensor_copy(out=wl16[:], in_=wl_sb[:])
            norm_sb = cpool.tile([128, n_slice_tiles], F32)
            nc.sync.dma_start(out=norm_sb[:], in_=norm_cols[:, :])
            src_sb = cpool.tile([128, total_e // 16], I16)
            nc.sync.dma_start(out=src_sb[:], in_=src_idx[:, :])
            dc_sb = cpool.tile([128, total_e // 128], F32)
            nc.sync.dma_start(out=dc_sb[:], in_=dst_rel[:, :])

            # ------- precompute hW/relW tables on PE (h^T fp16 uploaded) -------
            def xw_group(src_t, col0, nt, dst_dram, row0):
                ht4 = inpool.tile([128, 4 * D], F16, tag="ht4")
                nc.sync.dma_start(out=ht4[:, 0:nt * D],
                                  in_=src_t[:, col0:col0 + nt * D])
                sw4 = swpool.tile([128, 4, D], F16, tag="sw4")
                for b in range(nt):
                    pw = pwpool.tile([128, D], F32, tag="pw")
                    nc.tensor.matmul(out=pw[:], lhsT=ht4[:, b * D:(b + 1) * D],
                                     rhs=w16[:], start=True, stop=True)
                    nc.vector.tensor_copy(out=sw4[:, b, :], in_=pw[:])
                dst_ap = dst_dram[row0:row0 + nt * 128, :]
                nc.sync.dma_start(out=dst_ap.rearrange("(b p) d -> p b d", p=128),
                                  in_=sw4[:, 0:nt, :])

            # relW stays in SBUF (fp16) for the C^T @ relW matmuls
            relw16 = cpool.tile([128, n_r_tiles, D], F16)
            for rt in range(n_r_tiles):
                rtile = inpool.tile([128, D], F16, tag="rt16")
                nc.sync.dma_start(out=rtile[:], in_=rel_t[:, rt * D:(rt + 1) * D])
                pw = pwpool.tile([128, D], F32, tag="pw")
                nc.tensor.matmul(out=pw[:], lhsT=rtile[:], rhs=w16[:],
                                 start=True, stop=True)
                nc.vector.tensor_copy(out=relw16[:, rt, :], in_=pw[:])

            n_lo_tiles = cfg.split // 128
            for t0 in range(0, n_lo_tiles, 4):
                xw_group(h_t, t0 * 128, min(4, n_lo_tiles - t0), tab_lo, t0 * 128)
            for t0 in range(n_lo_tiles, n_h_tiles, 4):
                nt = min(4, n_h_tiles - t0)
                xw_group(h_t, t0 * 128, nt, tab_hi, t0 * 128 - cfg.split)

            # ------- per dst-tile: gather + selector matmuls + fused finish -------
            qctr = [0]

            def nextq():
                qctr[0] += 1
                return qctr[0] % 4

            for t in range(n_slice_tiles):
                base = t * seg
                st16 = segpool.tile([128, n_ktiles, D], F16, tag="seg")
                for part_base, part_len, tab in ((0, na, tab_lo), (na, nb, tab_hi)):
                    for c0 in range(0, part_len, GCHUNK):
                        n = min(GCHUNK, part_len - c0)
                        o = base + part_base + c0
                        nc.gpsimd.dma_gather(
                            out_ap=st16[:, (part_base + c0) // 128:(part_base + c0 + n) // 128, :],
                            in_ap=tab[:, :], idxs_ap=src_sb[:, o // 16:(o + n) // 16],
                            num_idxs=n, num_idxs_reg=n, elem_size=D, queue_num=nextq())

                agg = agpool.tile([128, D], F32, tag="agg")
                nmm = n_ktiles + n_r_tiles
                sel = selpool.tile([128, n_ktiles, 128], F16, tag="sel")
                dc_col = dc_sb[:, base // 128:base // 128 + n_ktiles]
                nc.vector.tensor_tensor(
                    out=sel[:], in0=iota_big[:],
                    in1=dc_col.rearrange("p (c o) -> p c o", o=1).broadcast_to(
                        [128, n_ktiles, 128]),
                    op=mybir.AluOpType.is_equal)
                k = 0
                for kt in range(n_ktiles):
                    nc.tensor.matmul(out=agg[:], lhsT=sel[:, kt, :], rhs=st16[:, kt, :],
                                     start=(k == 0), stop=(k == nmm - 1))
                    k += 1
                # rel contribution: C^T.T @ relW accumulated into the same PSUM
                ct = fpool.tile([128, n_r_tiles, 128], F16, tag="ct_ld")
                nc.sync.dma_start(
                    out=ct[:],
                    in_=ct_hist[:, t * 128:(t + 1) * 128].rearrange(
                        "(rt r) d -> r rt d", r=128))
                for rt in range(n_r_tiles):
                    nc.tensor.matmul(out=agg[:], lhsT=ct[:, rt, :], rhs=relw16[:, rt, :],
                                     start=(k == 0), stop=(k == nmm - 1))
                    k += 1

                # loop message (h_slice^T fp16 uploaded)
                hst = inpool.tile([128, D], F16, tag="hs")
                nc.sync.dma_start(out=hst[:], in_=hs_t[:, t * 128:(t + 1) * 128])
                pl = pwpool.tile([128, D], F32, tag="pw")
                nc.tensor.matmul(out=pl[:], lhsT=hst[:], rhs=wl16[:], start=True, stop=True)

                m = fpool.tile([128, D], F32)
                nc.vector.tensor_scalar(out=m[:], in0=agg[:], scalar1=norm_sb[:, t:t + 1],
                                        scalar2=None, op0=mybir.AluOpType.mult)
                nc.vector.tensor_add(out=m[:], in0=m[:], in1=pl[:])
                nc.vector.tensor_scalar_max(out=m[:], in0=m[:], scalar1=0.0)
                nc.sync.dma_start(out=out[t * 128:(t + 1) * 128, :], in_=m[:])

            if debug_outputs:
                def dbg_copy(src, dst, ntiles):
                    for tt in range(ntiles):
                        dd = fpool.tile([128, D], F16, tag="dbg")
                        nc.sync.dma_start(out=dd[:], in_=src[tt * 128:(tt + 1) * 128, :])
                        nc.sync.dma_start(out=dst[tt * 128:(tt + 1) * 128, :], in_=dd[:])
                dbg_copy(tab_lo, dbg_lo, cfg.split // 128)
                dbg_copy(tab_hi, dbg_hi, (cfg.n_pad - cfg.split) // 128)

    nc.compile()
    return nc


def _wrap16(vals: np.ndarray, pad_len: int, pad_val: int) -> np.ndarray:
    a = np.full(pad_len, pad_val, dtype=np.int16)
    a[:len(vals)] = vals.astype(np.int16)
    w16 = a.reshape(pad_len // 16, 16).T
    return np.tile(w16, (8, 1)).copy()


def prep_inputs(cfg: Cfg, h, norm, rel_emb, w_nb, w_loop, edge_src, edge_dst, edge_type):
    h = np.asarray(h, np.float32)
    norm = np.asarray(norm, np.float32).reshape(-1)
    rel_emb = np.asarray(rel_emb, np.float32)
    edge_src = np.asarray(edge_src, np.int64)
    edge_dst = np.asarray(edge_dst, np.int64)
    edge_type = np.asarray(edge_type, np.int64)

    na, nb = cfg.seg_size("A"), cfg.seg_size("B")
    seg = na + nb
    n_slice_tiles = cfg.np_pad // 128
    total_e = seg * n_slice_tiles

    h_pad = np.zeros((cfg.n_pad, cfg.d), np.float32)
    h_pad[:cfg.n_nodes] = h
    r_pad = np.zeros((cfg.r_pad, cfg.d), np.float32)
    r_pad[:cfg.n_rels] = rel_emb
    h_t = np.ascontiguousarray(h_pad.T.astype(np.float16))
    rel_t = np.ascontiguousarray(r_pad.T.astype(np.float16))

    in_maps = []
    for c in range(cfg.n_cores):
        lo, hi = c * cfg.np_core, (c + 1) * cfg.np_core
        sel = (edge_dst >= lo) & (edge_dst < hi)
        src_c, dst_c, typ_c = edge_src[sel], edge_dst[sel] - lo, edge_type[sel]

        src_stream = np.zeros(total_e, np.int64)
        dc_stream = np.full(total_e, -1.0, np.float32)
        dtile = dst_c // 128
        in_a = src_c < cfg.split
        for t in range(n_slice_tiles):
            tm = dtile == t
            for part0, plen, pm, soff in ((0, na, tm & in_a, 0),
                                          (na, nb, tm & ~in_a, cfg.split)):
                s = src_c[pm] - soff
                d = dst_c[pm] - t * 128
                assert len(s) <= plen, (c, t, len(s), plen)
                o = np.argsort(s, kind="stable")
                base = t * seg + part0
                src_stream[base:base + len(s)] = s[o]
                dc_stream[base:base + len(s)] = d[o]

        hsl = np.zeros((cfg.np_pad, cfg.d), np.float32)
        hsl[:cfg.np_core] = h[lo:hi]
        hsl_t = np.ascontiguousarray(hsl.T.astype(np.float16))
        ntmp = np.zeros(cfg.np_pad, np.float32)
        ntmp[:cfg.np_core] = norm[lo:hi]
        ncol = np.ascontiguousarray(ntmp.reshape(n_slice_tiles, cfg.d).T)

        ct = np.zeros((cfg.r_pad, cfg.np_pad), np.float32)
        np.add.at(ct, (typ_c, dst_c), 1.0)

        in_maps.append({
            "h_t": h_t, "rel_t": rel_t,
            "w_nb": np.asarray(w_nb, np.float32), "w_loop": np.asarray(w_loop, np.float32),
            "hs_t": hsl_t, "norm_cols": ncol,
            "ct_hist": ct.astype(np.float16),
            "src_idx": _wrap16(src_stream, total_e, 0),
            "dst_rel": np.ascontiguousarray(
                dc_stream.reshape(total_e // 128, 128).T),
        })
    return in_maps


_CACHED = {}


def _get_program(cfg: Cfg):
    if cfg not in _CACHED:
        _CACHED[cfg] = build_program(cfg)
    return _CACHED[cfg]


LAST_RESULTS = None


def kernel(h, norm, rel_emb, W_neighbor, loop_weight, edge_src, edge_dst, edge_type):
    cfg = CFG
    nc = _get_program(cfg)
    in_maps = prep_inputs(cfg, h, norm, rel_emb, W_neighbor, loop_weight,
                          edge_src, edge_dst, edge_type)
    trace = os.environ.get("KERNEL_TRACE", "0") == "1"
    res = run_bass_kernel_spmd(nc, in_maps, list(range(cfg.n_cores)), trace=trace)
    global LAST_RESULTS
    LAST_RESULTS = res
    outs = [res.results[c]["out"][:cfg.np_core] for c in range(cfg.n_cores)]
    return np.concatenate(outs, axis=0).astype(np.float32)


# revision 42
# speedup vs baseline: 1.2786x; 1.0005x over previous
"""CompGCN layer kernel for 8 Trainium2 NeuronCores.

Strategy (dst-sharded, gather + selector-matmul aggregation, no collectives):
  - Each core owns 6250 destination nodes and receives exactly the edges
    whose dst falls in its range (host bucketing).
  - (h[src] + rel[type]) @ W == (h@W)[src] + (rel@W)[type]. Each core builds
    hW = h @ W_neighbor (fp16 table, replicated) on the tensor engine.
  - Edges are grouped per dst-tile (128 dst nodes) into two statically-sized
    segments: A (src < 32768) and B (src >= 32768, local idx) — int16 gather
    indices can't span 50048 rows. GPSIMD dma_gather pulls hW[src] for each
    segment (edge-major fp16 tiles).
  - Aggregation is a matmul: for each 128-edge K-tile, a one-hot selector
    S[e, j] = (dst_rel_e == j) (built on DVE via iota + tensor_scalar
    is_equal) is the stationary operand; msg tiles are moving; PSUM [dst, d]
    accumulates all K-tiles of the dst-tile. Pad edges carry dst_rel = -1 so
    their selector column is all-zero — pads are free.
  - rel contribution folds in exactly as C^T @ relW (C = per-core dst x type
    count histogram, host-built, fp16-exact) accumulated into the same PSUM.
  - Fused final: out = relu((agg) * norm + h_slice @ loop_weight) per tile.
  - dma_gather HW limits: <= 1024 idxs per op is the empirically stable size
    (the SWDGE desc rings overflow beyond that; crash, not slowdown).
"""

import os
import math
import numpy as np
from dataclasses import dataclass
from contextlib import ExitStack

from concourse import bacc, bass, mybir, tile
from concourse.bass_utils import run_bass_kernel_spmd
from concourse.masks import make_identity

F32 = mybir.dt.float32
F16 = mybir.dt.float16
I16 = mybir.dt.int16

GCHUNK = int(os.environ.get("KERNEL_GCHUNK", "1024"))  # idxs per dma_gather


@dataclass(frozen=True)
class Cfg:
    n_nodes: int = 50000
    d: int = 128
    n_rels: int = 500
    n_cores: int = 8
    split: int = 32768
    n_edges: int = 600000

    @property
    def np_core(self):
        return self.n_nodes // self.n_cores

    @property
    def np_pad(self):
        return ((self.np_core + 1 + 127) // 128) * 128

    @property
    def n_pad(self):
        return ((self.n_nodes + 127) // 128) * 128

    @property
    def r_pad(self):
        return ((self.n_rels + 127) // 128) * 128

    def seg_size(self, table: str) -> int:
        """Static per-dst-tile segment size (multiple of 128) for stream A/B.

        Edge count of one dst-tile (128 dsts) in one stream is ~Poisson(mean);
        size at mean + 6.5 sigma, rounded up to 128."""
        lam_node = self.n_edges / self.n_nodes
        share = (self.split / self.n_nodes if table == "A"
                 else (self.n_nodes - self.split) / self.n_nodes)
        mean = 128 * lam_node * share
        sz = mean + 6.5 * math.sqrt(mean) + 16
        return int(math.ceil(sz / 128.0)) * 128


CFG = Cfg()


def build_program(cfg: Cfg, debug_outputs: bool = False):
    nc = bacc.Bacc("TRN2", target_bir_lowering=False, debug=False, num_swdge_queues=4)
    D = cfg.d
    na, nb = cfg.seg_size("A"), cfg.seg_size("B")
    seg = na + nb                       # edges per dst-tile segment
    n_slice_tiles = cfg.np_pad // 128
    n_h_tiles = cfg.n_pad // 128
    n_r_tiles = cfg.r_pad // 128
    total_e = seg * n_slice_tiles       # padded edge stream per core
    n_ktiles = seg // 128

    h_t = nc.dram_tensor("h_t", [D, cfg.n_pad], F16, kind="ExternalInput")
    rel_t = nc.dram_tensor("rel_t", [D, cfg.r_pad], F16, kind="ExternalInput")
    w_nb = nc.dram_tensor("w_nb", [D, D], F32, kind="ExternalInput")
    w_loop = nc.dram_tensor("w_loop", [D, D], F32, kind="ExternalInput")
    hs_t = nc.dram_tensor("hs_t", [D, cfg.np_pad], F16, kind="ExternalInput")
    norm_cols = nc.dram_tensor("norm_cols", [D, n_slice_tiles], F32, kind="ExternalInput")
    ct_pack = nc.dram_tensor("ct_pack", [cfg.np_pad // 128, 128, cfg.r_pad // 128, 128], F16, kind="ExternalInput")
    src_idx = nc.dram_tensor("src_idx", [128, total_e // 16], I16, kind="ExternalInput")
    dst_rel = nc.dram_tensor("dst_rel", [128, total_e // 128], F16, kind="ExternalInput")

    tab_lo = nc.dram_tensor("tab_lo", [cfg.split, D], F16)
    tab_hi = nc.dram_tensor("tab_hi", [cfg.n_pad - cfg.split, D], F16)
    out = nc.dram_tensor("out", [cfg.np_pad, D], F32, kind="ExternalOutput")
    if debug_outputs:
        dbg_lo = nc.dram_tensor("dbg_lo", [cfg.split, D], F16, kind="ExternalOutput")
        dbg_hi = nc.dram_tensor("dbg_hi", [cfg.n_pad - cfg.split, D], F16, kind="ExternalOutput")

    with tile.TileContext(nc) as tc:
        with ExitStack() as ex:
            cpool = ex.enter_context(tc.tile_pool(name="const", bufs=1))
            inpool = ex.enter_context(tc.tile_pool(name="ld", bufs=4))
            pwpool = ex.enter_context(tc.tile_pool(name="psW", bufs=3, space="PSUM"))
            agpool = ex.enter_context(tc.tile_pool(name="psA", bufs=4, space="PSUM"))
            swpool = ex.enter_context(tc.tile_pool(name="sbW", bufs=4))
            segpool = ex.enter_context(tc.tile_pool(name="seg", bufs=6))
            selpool = ex.enter_context(tc.tile_pool(name="sel", bufs=6))
            fpool = ex.enter_context(tc.tile_pool(name="fin", bufs=4))

            # ------- constants -------
            iota_big = cpool.tile([128, n_ktiles, 128], F16)
            nc.gpsimd.iota(iota_big[:], pattern=[[0, n_ktiles], [1, 128]], base=0,
                           channel_multiplier=0,
                           allow_small_or_imprecise_dtypes=True)
            w_sb = cpool.tile([128, D], F32)
            nc.sync.dma_start(out=w_sb[:], in_=w_nb[:, :])
            w16 = cpool.tile([128, D], F16)
            nc.vector.tensor_copy(out=w16[:], in_=w_sb[:])
            wl_sb = cpool.tile([128, D], F32)
            nc.sync.dma_start(out=wl_sb[:], in_=w_loop[:, :])
            wl16 = cpool.tile([128, D], F16)
            nc.vector.tensor_copy(out=wl16[:], in_=wl_sb[:])
            norm_sb = cpool.tile([128, n_slice_tiles], F32)
            nc.sync.dma_start(out=norm_sb[:], in_=norm_cols[:, :])
            src_sb = cpool.tile([128, total_e // 16], I16)
            nc.sync.dma_start(out=src_sb[:], in_=src_idx[:, :])
            dc_sb = cpool.tile([128, total_e // 128], F16)
            nc.sync.dma_start(out=dc_sb[:], in_=dst_rel[:, :])

            # ------- precompute hW/relW tables on PE (h^T fp16 uploaded) -------
            def xw_group(src_t, col0, nt, dst_dram, row0):
                ht4 = inpool.tile([128, 4 * D], F16, tag="ht4")
                nc.sync.dma_start(out=ht4[:, 0:nt * D],
                                  in_=src_t[:, col0:col0 + nt * D])
                sw4 = swpool.tile([128, 4, D], F16, tag="sw4")
                for b in range(nt):
                    pw = pwpool.tile([128, D], F32, tag="pw")
                    nc.tensor.matmul(out=pw[:], lhsT=ht4[:, b * D:(b + 1) * D],
                                     rhs=w16[:], start=True, stop=True)
                    nc.vector.tensor_copy(out=sw4[:, b, :], in_=pw[:])
                dst_ap = dst_dram[row0:row0 + nt * 128, :]
                nc.sync.dma_start(out=dst_ap.rearrange("(b p) d -> p b d", p=128),
                                  in_=sw4[:, 0:nt, :])

            # relW stays in SBUF (fp16) for the C^T @ relW matmuls
            relw16 = cpool.tile([128, n_r_tiles, D], F16)
            for rt in range(n_r_tiles):
                rtile = inpool.tile([128, D], F16, tag="rt16")
                nc.sync.dma_start(out=rtile[:], in_=rel_t[:, rt * D:(rt + 1) * D])
                pw = pwpool.tile([128, D], F32, tag="pw")
                nc.tensor.matmul(out=pw[:], lhsT=rtile[:], rhs=w16[:],
                                 start=True, stop=True)
                nc.vector.tensor_copy(out=relw16[:, rt, :], in_=pw[:])

            n_lo_tiles = cfg.split // 128
            for t0 in range(0, n_lo_tiles, 4):
                xw_group(h_t, t0 * 128, min(4, n_lo_tiles - t0), tab_lo, t0 * 128)
            for t0 in range(n_lo_tiles, n_h_tiles, 4):
                nt = min(4, n_h_tiles - t0)
                xw_group(h_t, t0 * 128, nt, tab_hi, t0 * 128 - cfg.split)

            # ------- per dst-tile: gather + selector matmuls + fused finish -------
            qctr = [0]

            def nextq():
                qctr[0] += 1
                return qctr[0] % 4

            for t in range(n_slice_tiles):
                base = t * seg
                st16 = segpool.tile([128, n_ktiles, D], F16, tag="seg")
                for part_base, part_len, tab in ((0, na, tab_lo), (na, nb, tab_hi)):
                    for c0 in range(0, part_len, GCHUNK):
                        n = min(GCHUNK, part_len - c0)
                        o = base + part_base + c0
                        nc.gpsimd.dma_gather(
                            out_ap=st16[:, (part_base + c0) // 128:(part_base + c0 + n) // 128, :],
                            in_ap=tab[:, :], idxs_ap=src_sb[:, o // 16:(o + n) // 16],
                            num_idxs=n, num_idxs_reg=n, elem_size=D, queue_num=nextq())

                agg = agpool.tile([128, D], F32, tag="agg")
                nmm = n_ktiles + n_r_tiles
                sel = selpool.tile([128, n_ktiles, 128], F16, tag="sel")
                dc_col = dc_sb[:, base // 128:base // 128 + n_ktiles]
                nc.vector.tensor_tensor(
                    out=sel[:], in0=iota_big[:],
                    in1=dc_col.rearrange("p (c o) -> p c o", o=1).broadcast_to(
                        [128, n_ktiles, 128]),
                    op=mybir.AluOpType.is_equal)
                k = 0
                for kt in range(n_ktiles):
                    nc.tensor.matmul(out=agg[:], lhsT=sel[:, kt, :], rhs=st16[:, kt, :],
                                     start=(k == 0), stop=(k == nmm - 1))
                    k += 1
                # rel contribution: C^T.T @ relW accumulated into the same PSUM
                ct = fpool.tile([128, n_r_tiles, 128], F16, tag="ct_ld")
                nc.sync.dma_start(out=ct[:], in_=ct_pack[t, :, :, :])
                for rt in range(n_r_tiles):
                    nc.tensor.matmul(out=agg[:], lhsT=ct[:, rt, :], rhs=relw16[:, rt, :],
                                     start=(k == 0), stop=(k == nmm - 1))
                    k += 1

                # loop message (h_slice^T fp16 uploaded)
                hst = inpool.tile([128, D], F16, tag="hs")
                nc.sync.dma_start(out=hst[:], in_=hs_t[:, t * 128:(t + 1) * 128])
                pl = pwpool.tile([128, D], F32, tag="pw")
                nc.tensor.matmul(out=pl[:], lhsT=hst[:], rhs=wl16[:], start=True, stop=True)

                pls = fpool.tile([128, D], F32, tag="pls")
                nc.scalar.activation(out=pls[:], in_=pl[:],
                                     func=mybir.ActivationFunctionType.Copy)
                m = fpool.tile([128, D], F32)
                nc.vector.scalar_tensor_tensor(
                    out=m[:], in0=agg[:], scalar=norm_sb[:, t:t + 1], in1=pls[:],
                    op0=mybir.AluOpType.mult, op1=mybir.AluOpType.add)
                nc.scalar.activation(out=m[:], in_=m[:],
                                     func=mybir.ActivationFunctionType.Relu)
                nc.sync.dma_start(out=out[t * 128:(t + 1) * 128, :], in_=m[:])

            if debug_outputs:
                def dbg_copy(src, dst, ntiles):
                    for tt in range(ntiles):
                        dd = fpool.tile([128, D], F16, tag="dbg")
                        nc.sync.dma_start(out=dd[:], in_=src[tt * 128:(tt + 1) * 128, :])
                        nc.sync.dma_start(out=dst[tt * 128:(tt + 1) * 128, :], in_=dd[:])
                dbg_copy(tab_lo, dbg_lo, cfg.split // 128)
                dbg_copy(tab_hi, dbg_hi, (cfg.n_pad - cfg.split) // 128)

    nc.compile()
    return nc


def _wrap16(vals: np.ndarray, pad_len: int, pad_val: int) -> np.ndarray:
    a = np.full(pad_len, pad_val, dtype=np.int16)
    a[:len(vals)] = vals.astype(np.int16)
    w16 = a.reshape(pad_len // 16, 16).T
    return np.tile(w16, (8, 1)).copy()


def prep_inputs(cfg: Cfg, h, norm, rel_emb, w_nb, w_loop, edge_src, edge_dst, edge_type):
    h = np.asarray(h, np.float32)
    norm = np.asarray(norm, np.float32).reshape(-1)
    rel_emb = np.asarray(rel_emb, np.float32)
    edge_src = np.asarray(edge_src, np.int64)
    edge_dst = np.asarray(edge_dst, np.int64)
    edge_type = np.asarray(edge_type, np.int64)

    na, nb = cfg.seg_size("A"), cfg.seg_size("B")
    seg = na + nb
    n_slice_tiles = cfg.np_pad // 128
    total_e = seg * n_slice_tiles

    h_pad = np.zeros((cfg.n_pad, cfg.d), np.float32)
    h_pad[:cfg.n_nodes] = h
    r_pad = np.zeros((cfg.r_pad, cfg.d), np.float32)
    r_pad[:cfg.n_rels] = rel_emb
    h_t = np.ascontiguousarray(h_pad.T.astype(np.float16))
    rel_t = np.ascontiguousarray(r_pad.T.astype(np.float16))

    in_maps = []
    for c in range(cfg.n_cores):
        lo, hi = c * cfg.np_core, (c + 1) * cfg.np_core
        sel = (edge_dst >= lo) & (edge_dst < hi)
        src_c, dst_c, typ_c = edge_src[sel], edge_dst[sel] - lo, edge_type[sel]

        src_stream = np.zeros(total_e, np.int64)
        dc_stream = np.full(total_e, -1.0, np.float32)
        dtile = dst_c // 128
        in_a = src_c < cfg.split
        for t in range(n_slice_tiles):
            tm = dtile == t
            for part0, plen, pm, soff in ((0, na, tm & in_a, 0),
                                          (na, nb, tm & ~in_a, cfg.split)):
                s = src_c[pm] - soff
                d = dst_c[pm] - t * 128
                assert len(s) <= plen, (c, t, len(s), plen)
                o = np.argsort(s, kind="stable")
                base = t * seg + part0
                src_stream[base:base + len(s)] = s[o]
                dc_stream[base:base + len(s)] = d[o]

        hsl = np.zeros((cfg.np_pad, cfg.d), np.float32)
        hsl[:cfg.np_core] = h[lo:hi]
        hsl_t = np.ascontiguousarray(hsl.T.astype(np.float16))
        ntmp = np.zeros(cfg.np_pad, np.float32)
        ntmp[:cfg.np_core] = norm[lo:hi]
        ncol = np.ascontiguousarray(ntmp.reshape(n_slice_tiles, cfg.d).T)

        ct = np.zeros((cfg.r_pad, cfg.np_pad), np.float32)
        np.add.at(ct, (typ_c, dst_c), 1.0)
        # [dst_tile, r % 128, r // 128, dst % 128] contiguous per dst-tile
        ctp = np.ascontiguousarray(
            ct.reshape(cfg.r_pad // 128, 128, n_slice_tiles, 128)
            .transpose(2, 1, 0, 3)).astype(np.float16)

        in_maps.append({
            "h_t": h_t, "rel_t": rel_t,
            "w_nb": np.asarray(w_nb, np.float32), "w_loop": np.asarray(w_loop, np.float32),
            "hs_t": hsl_t, "norm_cols": ncol,
            "ct_pack": ctp,
            "src_idx": _wrap16(src_stream, total_e, 0),
            "dst_rel": np.ascontiguousarray(
                dc_stream.reshape(total_e // 128, 128).T).astype(np.float16),
        })
    return in_maps


_CACHED = {}


def _get_program(cfg: Cfg):
    if cfg not in _CACHED:
        _CACHED[cfg] = build_program(cfg)
    return _CACHED[cfg]


LAST_RESULTS = None


def kernel(h, norm, rel_emb, W_neighbor, loop_weight, edge_src, edge_dst, edge_type):
    cfg = CFG
    nc = _get_program(cfg)
    in_maps = prep_inputs(cfg, h, norm, rel_emb, W_neighbor, loop_weight,
                          edge_src, edge_dst, edge_type)
    trace = os.environ.get("KERNEL_TRACE", "0") == "1"
    res = run_bass_kernel_spmd(nc, in_maps, list(range(cfg.n_cores)), trace=trace)
    global LAST_RESULTS
    LAST_RESULTS = res
    outs = [res.results[c]["out"][:cfg.np_core] for c in range(cfg.n_cores)]
    return np.concatenate(outs, axis=0).astype(np.float32)
